# revision 1
# baseline (speedup 1.0000x reference)
"""Trainium2 Bass kernel for nn_KPLoss_377957122199.

Keypoint loss = alpha*cross_entropy + beta*smoothL1(kp) + delta*smoothL1(Procrustes rot)
              + epsilon*smoothL1(centers),  alpha,beta,delta,eps = 1,4,5,6

Data-parallel over 8 NeuronCores: batch 8192 -> 1024 per core. Each core
produces per-partition partial sums; host combines (weighted means).

Key device tricks:
  * smooth_l1 sums via  sum f(d) = 0.5*sum d^2 - 0.5*sum u^2 + sum u - N/2,
    u = max(|d|,1)  (one tensor_scalar(abs_max)+accum and two ACT Square+accum)
  * cross entropy without max-subtraction (logits are O(5)); one-hot mask via
    gpsimd is_equal against an iota tile; sum(l_y) via fused tensor_tensor_reduce
  * Procrustes rotation R = polar(H) via Frobenius-scaled Newton iteration
    (4 iters + 1 Newton-Schulz polish), batched over all sections as
    [128,160] elementwise planes.
"""

import sys
for _p in ("/opt/trn_rl_repo", "/root/.axon_site/_ro/trn_rl_repo"):
    if _p not in sys.path:
        sys.path.insert(0, _p)

from contextlib import ExitStack

import numpy as np
import ml_dtypes

import concourse.bass as bass
import concourse.bacc as bacc
import concourse.mybir as mybir
import concourse.tile as tile
from concourse.bass_utils import run_bass_kernel_spmd

FP32 = mybir.dt.float32
BF16 = mybir.dt.bfloat16
AX = mybir.AxisListType
OP = mybir.AluOpType
AF = mybir.ActivationFunctionType

N_CORES = 8
B, K, NS, SEC = 8192, 400, 20, 20
S = K // SEC                      # 20 sections per sample
BC = B // N_CORES                 # 1024 samples per core
NCH_KP = BC // 128                # 8 keypoint chunks of 128 samples
SECS = BC * S                     # 20480 sections per core
SFD = SECS // 128                 # 160 sections per partition
NCH_CE = 20                       # cross-entropy chunks
TOK = BC * K                      # 409600 tokens per core
T_CE = TOK // (NCH_CE * 128)      # 160 tokens per partition per chunk
N_KP = BC * K * 3                 # smooth-l1 element count (kp and rot)
N_CENT = BC * S * 3

# acc column map (fp32 [128, NACC] output)
# smooth_l1 sums use the identity  sum f(d) = 0.5*(sum d^2 - sum relu(|d|-1)^2)
C_LSE = 0                         # NCH_CE cols
C_LY = C_LSE + NCH_CE
C_KP = C_LY + NCH_CE              # 2*NCH_KP cols: d2, r2 per chunk
C_ROT = C_KP + 2 * NCH_KP
C_CENT = C_ROT + 2 * NCH_KP
NACC = C_CENT + 2 * NCH_KP


def _emit(ctx: ExitStack, tc: "tile.TileContext", aps: dict):
    nc = tc.nc
    pk, gk, lg, lb, out = aps["pk"], aps["gk"], aps["lg"], aps["lb"], aps["out"]

    io = ctx.enter_context(tc.tile_pool(name="io", bufs=2))
    work = ctx.enter_context(tc.tile_pool(name="work", bufs=2))
    pers = ctx.enter_context(tc.tile_pool(name="pers", bufs=1))
    polp = ctx.enter_context(tc.tile_pool(name="polar", bufs=1))
    cep = ctx.enter_context(tc.tile_pool(name="ce", bufs=2))

    acc = pers.tile([128, NACC], FP32, tag="acc", name="acc")
    # every column is written exactly once by an accum_out; no memset needed
    neg1 = pers.tile([128, 1], FP32, tag="neg1", name="neg1")
    nc.gpsimd.memset(neg1[:], -1.0)

    sp_all = pers.tile([128, NCH_KP * 60], FP32, tag="sp", name="sp")   # per-chunk d-major point sums
    sg_all = pers.tile([128, NCH_KP * 60], FP32, tag="sg", name="sg")
    H = [[pers.tile([128, SFD], FP32, tag=f"H{i}{j}", name=f"H{i}{j}") for j in range(3)] for i in range(3)]

    # ---------------- phase 1: keypoint pass ----------------
    def kp_load_deint(c):
        pkc = io.tile([128, 1200], FP32, tag="pkc", name="pkc")
        gkc = io.tile([128, 1200], FP32, tag="gkc", name="gkc")
        nc.sync.dma_start(pkc[:], pk[c])
        nc.sync.dma_start(gkc[:], gk[c])
        pb = io.tile([128, 1200], BF16, tag="pb", name="pb")
        gb = io.tile([128, 1200], BF16, tag="gb", name="gb")
        # interleaved (s k d) -> d-major (d s k), cast to bf16
        for src, dst in ((pkc, pb), (gkc, gb)):
            v = src[:].rearrange("p (s k d) -> p d s k", s=SEC, k=SEC, d=3)
            for d in range(3):
                nc.gpsimd.tensor_copy(
                    dst[:, d * 400:(d + 1) * 400].rearrange("p (s k) -> p s k", s=SEC),
                    v[:, d],
                )
        return pb, gb

    def smooth_l1_acc(dt_tile, fd, col_base, c, u_tag):
        """sum d^2 and sum relu(|d|-1)^2 for this chunk -> two acc columns (ACT only)."""
        a = work.tile([128, fd], BF16, tag=u_tag, name=u_tag)
        nc.scalar.activation(a[:], dt_tile[:], AF.Abs)
        r = work.tile([128, fd], BF16, tag=u_tag + "r", name=u_tag + "r")
        nc.scalar.activation(r[:], a[:], AF.Relu, bias=neg1[:])
        tr = work.tile([128, fd], BF16, tag=u_tag + "tr", name=u_tag + "tr")
        nc.scalar.activation(tr[:], dt_tile[:], AF.Square,
                             accum_out=acc[:, col_base + c: col_base + c + 1])
        nc.scalar.activation(tr[:], r[:], AF.Square,
                             accum_out=acc[:, col_base + NCH_KP + c: col_base + NCH_KP + c + 1])

    for c in range(NCH_KP):
        pb, gb = kp_load_deint(c)
        # keypoint smooth-l1
        dt = work.tile([128, 1200], BF16, tag="kpd", name="kpd")
        nc.vector.tensor_sub(dt[:], pb[:], gb[:])
        smooth_l1_acc(dt, 1200, C_KP, c, "slu")
        # per-(d,section) point sums (sum over k): [128,3,20,20] -> [128,3,20]
        for src, dst in ((pb, sp_all), (gb, sg_all)):
            nc.vector.tensor_reduce(
                dst[:, c * 60:(c + 1) * 60].rearrange("p (d s) -> p d s", d=3),
                src[:].rearrange("p (d s k) -> p d s k", d=3, s=SEC, k=SEC),
                axis=AX.X, op=OP.add,
            )
        # center loss: mean diff = (sp-sg)/SEC
        dc = work.tile([128, 60], FP32, tag="centd", name="centd")
        nc.vector.tensor_sub(dc[:], sp_all[:, c * 60:(c + 1) * 60], sg_all[:, c * 60:(c + 1) * 60])
        dcm = work.tile([128, 60], BF16, tag="centdm", name="centdm")
        nc.vector.tensor_scalar(dcm[:], dc[:], 1.0 / SEC, None, OP.mult)
        smooth_l1_acc(dcm, 60, C_CENT, c, "slu")
        # raw H_ij = sum_k G_ki P_kj  (per section)
        eng = [nc.vector, nc.gpsimd]
        for i in range(3):
            for j in range(3):
                pr = work.tile([128, 400], BF16, tag=f"hprod{(i * 3 + j) % 2}", name=f"hprod{(i * 3 + j) % 2}")
                eng[(i * 3 + j) % 2].tensor_mul(
                    pr[:], gb[:, i * 400:(i + 1) * 400], pb[:, j * 400:(j + 1) * 400])
                nc.vector.tensor_reduce(
                    H[i][j][:, c * 20:(c + 1) * 20],
                    pr[:].rearrange("p (s k) -> p s k", s=SEC),
                    axis=AX.X, op=OP.add,
                )

    # H correction: H_ij -= (1/SEC) * sg_i * sp_j   (views over all chunks)
    sps = pers.tile([128, NCH_KP * 60], FP32, tag="sps", name="sps")
    nc.vector.tensor_scalar(sps[:], sp_all[:], 1.0 / SEC, None, OP.mult)

    def dsum_view(t, i):
        # [128, (chunk, d, s)] -> fixed d=i -> [128, chunk, s] == [128, SFD]
        return t[:].rearrange("p (c d s) -> p d c s", c=NCH_KP, d=3, s=S)[:, i]

    for i in range(3):
        for j in range(3):
            m = work.tile([128, SFD], FP32, tag="hc", name="hc")
            nc.vector.tensor_mul(m[:], dsum_view(sg_all, i), dsum_view(sps, j))
            nc.vector.tensor_sub(
                H[i][j][:].rearrange("p (c s) -> p c s", c=NCH_KP),
                H[i][j][:].rearrange("p (c s) -> p c s", c=NCH_KP),
                m[:].rearrange("p (c s) -> p c s", c=NCH_KP),
            )

    # ---------------- polar decomposition: R = polar(H) ----------------
    X = H  # in place; H not needed afterwards
    rr = [0]
    engs = [nc.vector, nc.gpsimd]

    def tt(op, out, a, b):
        engs[rr[0] % 2].tensor_tensor(out[:], a[:], b[:], op)
        rr[0] += 1

    def cof_det(Xc):
        C = [[polp.tile([128, SFD], FP32, tag=f"cof{i}{j}", name=f"cof{i}{j}") for j in range(3)] for i in range(3)]
        t1 = polp.tile([128, SFD], FP32, tag="cdt1", name="cdt1")
        idx = [(1, 2), (2, 0), (0, 1)]
        for i in range(3):
            for j in range(3):
                (a, b_), (cc, dd) = idx[i], idx[j]
                # cof[i][j] = X[a][cc]*X[b_][dd] - X[a][dd]*X[b_][cc]
                m1 = polp.tile([128, SFD], FP32, tag="cm1", name="cm1")
                m2 = polp.tile([128, SFD], FP32, tag="cm2", name="cm2")
                tt(OP.mult, m1, Xc[a][cc], Xc[b_][dd])
                tt(OP.mult, m2, Xc[a][dd], Xc[b_][cc])
                tt(OP.subtract, C[i][j], m1, m2)
        det = polp.tile([128, SFD], FP32, tag="det", name="det")
        nc.vector.tensor_mul(det[:], Xc[0][0][:], C[0][0][:])
        nc.vector.tensor_mul(t1[:], Xc[0][1][:], C[0][1][:])
        nc.vector.tensor_add(det[:], det[:], t1[:])
        nc.vector.tensor_mul(t1[:], Xc[0][2][:], C[0][2][:])
        nc.vector.tensor_add(det[:], det[:], t1[:])
        return C, det

    def frob2(M, tag):
        n2 = polp.tile([128, SFD], FP32, tag=tag)
        t = polp.tile([128, SFD], FP32, tag=tag + "t")
        nc.vector.tensor_mul(n2[:], M[0][0][:], M[0][0][:])
        for i in range(3):
            for j in range(3):
                if i == 0 and j == 0:
                    continue
                eng = engs[(i * 3 + j) % 2]
                eng.tensor_mul(t[:], M[i][j][:], M[i][j][:])
                nc.vector.tensor_add(n2[:], n2[:], t[:])
        return n2

    for it in range(4):
        C, det = cof_det(X)
        nX2 = frob2(X, "nx2")
        nC2 = frob2(C, "nc2")
        # zeta = (nC2/nX2)^(1/4) / sqrt(|det|)
        q = polp.tile([128, SFD], FP32, tag="q", name="q")
        qr = polp.tile([128, SFD], FP32, tag="qr", name="qr")
        nc.vector.reciprocal(qr[:], nX2[:])
        nc.vector.tensor_mul(q[:], nC2[:], qr[:])
        nc.scalar.activation(q[:], q[:], AF.Sqrt)
        nc.scalar.activation(q[:], q[:], AF.Sqrt)
        da = polp.tile([128, SFD], FP32, tag="da", name="da")
        nc.scalar.activation(da[:], det[:], AF.Abs)
        nc.scalar.activation(da[:], da[:], AF.Sqrt)
        dr = polp.tile([128, SFD], FP32, tag="dr", name="dr")
        nc.vector.reciprocal(dr[:], da[:])
        zeta = polp.tile([128, SFD], FP32, tag="zeta", name="zeta")
        nc.vector.tensor_mul(zeta[:], q[:], dr[:])
        # X' = 0.5*zeta*X + (0.5/(zeta*det)) * C
        hz = polp.tile([128, SFD], FP32, tag="hz", name="hz")
        nc.vector.tensor_scalar(hz[:], zeta[:], 0.5, None, OP.mult)
        u = polp.tile([128, SFD], FP32, tag="uu", name="uu")
        nc.vector.tensor_mul(u[:], zeta[:], det[:])
        w = polp.tile([128, SFD], FP32, tag="ww", name="ww")
        nc.vector.reciprocal(w[:], u[:])
        nc.vector.tensor_scalar(w[:], w[:], 0.5, None, OP.mult)
        Xn = [[polp.tile([128, SFD], FP32, tag=f"X{i}{j}", name=f"X{i}{j}") for j in range(3)] for i in range(3)]
        for i in range(3):
            for j in range(3):
                a = polp.tile([128, SFD], FP32, tag="ua", name="ua")
                b_ = polp.tile([128, SFD], FP32, tag="ub", name="ub")
                tt(OP.mult, a, X[i][j], hz)
                tt(OP.mult, b_, C[i][j], w)
                tt(OP.add, Xn[i][j], a, b_)
        X = Xn

    # one Newton-Schulz polish: X = X(1.5 I - 0.5 X^T X)
    Y = [[None] * 3 for _ in range(3)]
    for i in range(3):
        for j in range(i, 3):
            y = polp.tile([128, SFD], FP32, tag=f"Y{i}{j}", name=f"Y{i}{j}")
            t = polp.tile([128, SFD], FP32, tag="yt", name="yt")
            nc.vector.tensor_mul(y[:], X[0][i][:], X[0][j][:])
            for k in (1, 2):
                engs[k % 2].tensor_mul(t[:], X[k][i][:], X[k][j][:])
                nc.vector.tensor_add(y[:], y[:], t[:])
            Y[i][j] = Y[j][i] = y
    W = [[None] * 3 for _ in range(3)]
    for i in range(3):
        for j in range(i, 3):
            w_ = polp.tile([128, SFD], FP32, tag=f"W{i}{j}", name=f"W{i}{j}")
            if i == j:
                nc.vector.tensor_scalar(w_[:], Y[i][j][:], -0.5, 1.5, OP.mult, OP.add)
            else:
                nc.vector.tensor_scalar(w_[:], Y[i][j][:], -0.5, None, OP.mult)
            W[i][j] = W[j][i] = w_
    R = [[polp.tile([128, SFD], FP32, tag=f"R{i}{j}", name=f"R{i}{j}") for j in range(3)] for i in range(3)]
    for i in range(3):
        for j in range(3):
            t = polp.tile([128, SFD], FP32, tag="rt", name="rt")
            nc.vector.tensor_mul(R[i][j][:], X[i][0][:], W[0][j][:])
            for k in (1, 2):
                engs[k % 2].tensor_mul(t[:], X[i][k][:], W[k][j][:])
                nc.vector.tensor_add(R[i][j][:], R[i][j][:], t[:])

    # v_j (per section) = (1/SEC) * (sum_i sp_i R_ij - sg_j)
    v_all = pers.tile([128, 3 * SFD], FP32, tag="vall", name="vall")
    for j in range(3):
        vj = v_all[:, j * SFD:(j + 1) * SFD]
        t = work.tile([128, SFD], FP32, tag="vt", name="vt")
        nc.vector.tensor_mul(vj, dsum_view(sp_all, 0), R[0][j][:])
        for i in (1, 2):
            engs[i % 2].tensor_mul(t[:], dsum_view(sp_all, i), R[i][j][:])
            nc.vector.tensor_add(vj, vj, t[:])
        nc.vector.tensor_sub(vj, vj, dsum_view(sg_all, j))
        nc.vector.tensor_scalar(vj, vj, 1.0 / SEC, None, OP.mult)

    # ---------------- phase 3: rotation residual ----------------
    for c in range(NCH_KP):
        pb, gb = kp_load_deint(c)
        # expand per-section R, v over k (broadcast) in bf16
        Re = [[work.tile([128, 400], BF16, tag=f"Re{i}{j}", name=f"Re{i}{j}", bufs=1) for j in range(3)] for i in range(3)]
        for i in range(3):
            for j in range(3):
                nc.gpsimd.tensor_copy(
                    Re[i][j][:].rearrange("p (s k) -> p s k", s=S),
                    R[i][j][:, c * 20:(c + 1) * 20].unsqueeze(2).broadcast_to([128, S, SEC]),
                )
        rfull = work.tile([128, 1200], BF16, tag="rfull", name="rfull")
        for j in range(3):
            ve = work.tile([128, 400], BF16, tag="ve", name="ve")
            nc.gpsimd.tensor_copy(
                ve[:].rearrange("p (s k) -> p s k", s=S),
                v_all[:, j * SFD + c * 20: j * SFD + (c + 1) * 20]
                .unsqueeze(2).broadcast_to([128, S, SEC]),
            )
            rj = rfull[:, j * 400:(j + 1) * 400]
            t = work.tile([128, 400], BF16, tag="rt3", name="rt3")
            nc.vector.tensor_mul(rj, pb[:, 0:400], Re[0][j][:])
            for i in (1, 2):
                nc.vector.tensor_mul(t[:], pb[:, i * 400:(i + 1) * 400], Re[i][j][:])
                nc.vector.tensor_add(rj, rj, t[:])
            nc.vector.tensor_sub(rj, rj, gb[:, j * 400:(j + 1) * 400])
            nc.vector.tensor_sub(rj, rj, ve[:])
        smooth_l1_acc(rfull, 1200, C_ROT, c, "slu")

    # ---------------- cross entropy ----------------
    iota = pers.tile([128, T_CE * NS], BF16, tag="iota", name="iota")
    nc.gpsimd.iota(iota[:], pattern=[[0, T_CE], [1, NS]], base=0,
                   channel_multiplier=0, allow_small_or_imprecise_dtypes=True)
    for c in range(NCH_CE):
        lgc = cep.tile([128, T_CE * NS], FP32, tag="lgc", name="lgc")
        nc.sync.dma_start(lgc[:], lg[c])
        lbc = cep.tile([128, T_CE], BF16, tag="lbc", name="lbc")
        nc.sync.dma_start(lbc[:], lb[c])
        ex = cep.tile([128, T_CE * NS], BF16, tag="ex", name="ex")
        nc.scalar.activation(ex[:], lgc[:], AF.Exp)
        s10 = cep.tile([128, T_CE * 10], BF16, tag="s10", name="s10")
        ex3 = ex[:].rearrange("p (t n) -> p t n", t=T_CE)
        nc.vector.tensor_add(
            s10[:].rearrange("p (t n) -> p t n", t=T_CE),
            ex3[:, :, 0:10], ex3[:, :, 10:20])
        se = cep.tile([128, T_CE], FP32, tag="se", name="se")
        nc.vector.tensor_reduce(
            se[:], s10[:].rearrange("p (t n) -> p t n", t=T_CE), axis=AX.X, op=OP.add)
        lt = cep.tile([128, T_CE], BF16, tag="lt", name="lt")
        nc.scalar.activation(lt[:], se[:], AF.Ln,
                             accum_out=acc[:, C_LSE + c: C_LSE + c + 1])
        lbe = cep.tile([128, T_CE * NS], BF16, tag="lbe", name="lbe", bufs=1)
        nc.gpsimd.tensor_copy(
            lbe[:].rearrange("p (t n) -> p t n", t=T_CE),
            lbc[:].unsqueeze(2).broadcast_to([128, T_CE, NS]))
        mask = cep.tile([128, T_CE * NS], BF16, tag="mask", name="mask", bufs=1)
        nc.vector.tensor_tensor(mask[:], lbe[:], iota[:], OP.is_equal)
        # sum l_y = sum ln(sum_j mask * exp(l))  (masked-exp keeps 2x bf16 modes)
        me = cep.tile([128, T_CE * NS], BF16, tag="me", name="me", bufs=1)
        nc.vector.tensor_mul(me[:], mask[:], ex[:])
        m10 = cep.tile([128, T_CE * 10], BF16, tag="m10", name="m10")
        me3 = me[:].rearrange("p (t n) -> p t n", t=T_CE)
        nc.vector.tensor_add(
            m10[:].rearrange("p (t n) -> p t n", t=T_CE),
            me3[:, :, 0:10], me3[:, :, 10:20])
        mse = cep.tile([128, T_CE], FP32, tag="mse", name="mse")
        nc.vector.tensor_reduce(
            mse[:], m10[:].rearrange("p (t n) -> p t n", t=T_CE), axis=AX.X, op=OP.add)
        lt2 = cep.tile([128, T_CE], BF16, tag="lt2", name="lt2")
        nc.scalar.activation(lt2[:], mse[:], AF.Ln,
                             accum_out=acc[:, C_LY + c: C_LY + c + 1])

    nc.sync.dma_start(out[:], acc[:])


_CACHE = {}


def _build():
    if "nc" in _CACHE:
        return _CACHE["nc"]
    nc = bacc.Bacc("TRN2", target_bir_lowering=False, debug=False,
                   enable_asserts=False, num_devices=N_CORES)
    aps = {
        "pk": nc.dram_tensor("pk", [NCH_KP, 128, 1200], FP32, kind="ExternalInput").ap(),
        "gk": nc.dram_tensor("gk", [NCH_KP, 128, 1200], FP32, kind="ExternalInput").ap(),
        "lg": nc.dram_tensor("lg", [NCH_CE, 128, T_CE * NS], FP32, kind="ExternalInput").ap(),
        "lb": nc.dram_tensor("lb", [NCH_CE, 128, T_CE], BF16, kind="ExternalInput").ap(),
        "out": nc.dram_tensor("out", [128, NACC], FP32, kind="ExternalOutput").ap(),
    }
    with tile.TileContext(nc) as tc:
        with ExitStack() as ctx:
            _emit(ctx, tc, aps)
    nc.compile()
    _CACHE["nc"] = nc
    return nc


def _shard_inputs(pred_keypoints, gt_keypoints, pred_section_logits, gt_section_label):
    pk = np.ascontiguousarray(pred_keypoints, dtype=np.float32).reshape(N_CORES, NCH_KP, 128, 1200)
    gk = np.ascontiguousarray(gt_keypoints, dtype=np.float32).reshape(N_CORES, NCH_KP, 128, 1200)
    lg = np.ascontiguousarray(pred_section_logits, dtype=np.float32).reshape(
        N_CORES, NCH_CE, 128, T_CE * NS)
    lb = np.ascontiguousarray(gt_section_label).reshape(N_CORES, NCH_CE, 128, T_CE).astype(
        ml_dtypes.bfloat16)
    return [
        {"pk": pk[i], "gk": gk[i], "lg": lg[i], "lb": lb[i]}
        for i in range(N_CORES)
    ]


def combine_accs(accs):
    """accs: list of [128, NACC] fp32 arrays (one per core) -> scalar loss."""
    tot = np.zeros(NACC, dtype=np.float64)
    for a in accs:
        tot += a.astype(np.float64).sum(axis=0)

    def sl1(base, n_per_chunk_elems):
        d2 = tot[base:base + NCH_KP].sum()
        r2 = tot[base + NCH_KP:base + 2 * NCH_KP].sum()
        return 0.5 * (d2 - r2)

    ce_sum = tot[C_LSE:C_LSE + NCH_CE].sum() - tot[C_LY:C_LY + NCH_CE].sum()
    kp_sum = sl1(C_KP, N_KP)
    rot_sum = sl1(C_ROT, N_KP)
    cent_sum = sl1(C_CENT, N_CENT)
    total = (1.0 * ce_sum / (B * K)
             + 4.0 * kp_sum / (B * K * 3)
             + 5.0 * rot_sum / (B * K * 3)
             + 6.0 * cent_sum / (B * S * 3))
    return np.float32(total)


def kernel(**inputs) -> np.ndarray:
    nc = _build()
    in_maps = _shard_inputs(**inputs)
    res = run_bass_kernel_spmd(nc, in_maps, list(range(N_CORES))).results
    return combine_accs([res[i]["out"] for i in range(N_CORES)])



# revision 23
# speedup vs baseline: 2.9175x; 2.9175x over previous
"""Trainium2 Bass kernel for nn_KPLoss_377957122199.

loss = 1*cross_entropy + 4*smoothL1(kp) + 5*smoothL1(Procrustes rot)
     + 6*smoothL1(section centers)

Data-parallel over 8 NeuronCores: batch 8192 -> 1024 per core; each core
emits per-partition partial sums, host combines.

Design notes (v2, instruction-count-optimized):
  * smooth_l1 sums via  sum f(d) = 0.5*sum d^2 - 0.5*sum u^2 + sum u - N/2,
    u = max(|d|,1):  ONE 4x-mode DVE tensor_scalar(abs_max)+accum and two
    ACT Square+accum per batch of elements.
  * keypoints deinterleaved (s k d)->(d s k) ONCE into persistent bf16
    SBUF tiles via 6 strided ACT copies per chunk; both the H-product pass
    and the rotation-residual pass read them (no second HBM load).
  * per-section 3x3 Gram matrices H via 9 bf16 2x DVE muls + an in-place
    pairwise add tree + one strided reduce per chunk.
  * polar decomposition R = polar(H) by Frobenius-scaled Newton iteration
    on a [128, 5,5,160] bf16 tile with duplicated rows/cols, so all 9
    cofactors come from THREE whole-matrix tensor_tensors on affine views.
    The zeta scaling runs in ln/exp domain so the only ACT functions the
    whole kernel needs (Copy/Square/Abs/Ln/Exp) live in ONE table set.
  * cross entropy: host swaps logits[t, y_t] <-> logits[t, 0] while
    sharding, so sum(l_y) is a strided-view reduce; lse = Ln(tree-summed
    exp) accumulated once over the whole core at the end.
  * CE chunks are emission-interleaved with the keypoint/polar phases so
    the logit DMA (the bulk of HBM traffic) prefetches throughout.
"""

import sys
for _p in ("/opt/trn_rl_repo", "/root/.axon_site/_ro/trn_rl_repo"):
    if _p not in sys.path:
        sys.path.insert(0, _p)

import math
from contextlib import ExitStack

import numpy as np

import concourse.bass as bass
import concourse.bacc as bacc
import concourse.mybir as mybir
import concourse.tile as tile
from concourse.bass_utils import run_bass_kernel_spmd

FP32 = mybir.dt.float32
BF16 = mybir.dt.bfloat16
AX = mybir.AxisListType
OP = mybir.AluOpType
AF = mybir.ActivationFunctionType

N_CORES = 8
B, K, NS, SEC = 8192, 400, 20, 20
S = K // SEC                      # 20 sections per sample
BC = B // N_CORES                 # 1024 samples per core
SPP = BC // 128                   # 8 samples per partition
SFD = SPP * S                     # 160 sections per partition
NCH_KP = 4                        # keypoint chunks
SPC = SPP // NCH_KP               # 2 samples per partition per chunk
FKP = SPC * K * 3                 # 2400 fp32 per partition per chunk
SCH = SPC * S                     # 40 sections per partition per chunk
NCH_CE = 20                       # cross-entropy chunks
TCE = BC * K // (NCH_CE * 128)    # 160 tokens per partition per chunk
FCE = TCE * NS                    # 3200 logits per partition per chunk
N_KP = BC * K * 3                 # smooth-l1 element counts (per core)
N_CENT = BC * S * 3

# acc column maps.  acc_v: DVE-written, acc_a: ACT-written (separate tiles
# so the two engines never touch the same SBUF region).
CV_LY = 0                         # NCH_CE cols
NACC_V = CV_LY + NCH_CE           # 20

CA_LSE = 0
CA_KPD2 = 1
CA_KPR2 = CA_KPD2 + NCH_KP
CA_ROTD2 = CA_KPR2 + NCH_KP
CA_ROTR2 = CA_ROTD2 + NCH_KP
CA_CTD2 = CA_ROTR2 + NCH_KP
CA_CTR2 = CA_CTD2 + 1
NACC_A = CA_CTR2 + 1              # 19

LN_HALF = float(math.log(0.5))
LN_FLOOR = float(math.log(1e-3))      # relative det floor: |det| >= 1e-3 ||X||^3
LN_SQRT3 = float(0.5 * math.log(3.0))


def _emit(ctx: ExitStack, tc: "tile.TileContext", aps: dict):
    nc = tc.nc
    pk, gk, lg = aps["pk"], aps["gk"], aps["lg"]
    out_v, out_a = aps["out_v"], aps["out_a"]

    pers = ctx.enter_context(tc.tile_pool(name="pers", bufs=1))
    io = ctx.enter_context(tc.tile_pool(name="io", bufs=2))
    wk = ctx.enter_context(tc.tile_pool(name="wk", bufs=2))
    pol = ctx.enter_context(tc.tile_pool(name="pol", bufs=1))
    cep = ctx.enter_context(tc.tile_pool(name="ce", bufs=2))

    acc_v = pers.tile([128, NACC_V], FP32, tag="accv", name="accv")
    acc_a = pers.tile([128, NACC_A], FP32, tag="acca", name="acca")
    pb = pers.tile([128, 3 * SFD * SEC], BF16, tag="pb", name="pb")    # [p,d,s,k]
    gb = pers.tile([128, 3 * SFD * SEC], BF16, tag="gb", name="gb")
    hsp = pers.tile([128, 15 * SFD], FP32, tag="hsp", name="hsp")      # 0-8 H, 9-11 sp, 12-14 sg
    xd = pers.tile([128, 25 * SFD], BF16, tag="xd", name="xd")         # [p,r5,c5,s]
    vv = pers.tile([128, 3 * SFD], BF16, tag="vv", name="vv")          # [p,j,s]
    se_all = pers.tile([128, NCH_CE * TCE], FP32, tag="se", name="se")

    pball = pb[:].rearrange("p (d s k) -> p d s k", d=3, s=SFD, k=SEC)
    gball = gb[:].rearrange("p (d s k) -> p d s k", d=3, s=SFD, k=SEC)
    hv = hsp[:].rearrange("p (n s) -> p n s", n=15)
    xdv = xd[:].rearrange("p (r c s) -> p r c s", r=5, c=5)
    X9 = xdv[:, 0:3, 0:3]                                              # [p,3,3,SFD]
    vvv = vv[:].rearrange("p (j s) -> p j s", j=3)

    c_eps = pers.tile([128, 1], FP32, tag="ceps", name="ceps")
    nc.gpsimd.memset(c_eps[:], 1e-12)
    c_lnh = pers.tile([128, 1], FP32, tag="clnh", name="clnh")
    nc.gpsimd.memset(c_lnh[:], LN_HALF)
    neg1 = pers.tile([128, 1], FP32, tag="neg1", name="neg1")
    nc.gpsimd.memset(neg1[:], -1.0)
    c_sq3 = pers.tile([128, 1], FP32, tag="csq3", name="csq3")
    nc.gpsimd.memset(c_sq3[:], LN_SQRT3)

    junk = pers.tile([128, FKP], BF16, tag="junk", name="junk")

    # ---------------- cross-entropy chunk (emission-interleaved) ----------
    def ce_chunk(ci):
        lgc = cep.tile([128, FCE], FP32, tag="lgc", name="lgc")
        nc.sync.dma_start(lgc[:], lg[ci])
        # sum of slot-0 logits (host swapped the label logit into slot 0)
        lyv = lgc[:].rearrange("p (t c) -> p c t", c=NS)[:, 0]
        nc.vector.tensor_reduce(acc_v[:, CV_LY + ci: CV_LY + ci + 1], lyv,
                                axis=AX.X, op=OP.add)
        ex = cep.tile([128, FCE], BF16, tag="ex", name="ex")
        nc.scalar.activation(ex[:], lgc[:], AF.Exp)
        exv = ex[:].rearrange("p (t c) -> p t c", t=TCE)
        nc.vector.tensor_tensor(exv[:, :, 0:10], exv[:, :, 0:10],
                                exv[:, :, 10:20], OP.add)
        nc.vector.tensor_tensor(exv[:, :, 0:5], exv[:, :, 0:5],
                                exv[:, :, 5:10], OP.add)
        nc.vector.tensor_reduce(se_all[:, ci * TCE:(ci + 1) * TCE],
                                exv[:, :, 0:5], axis=AX.X, op=OP.add)

    # ---------------- phase 1: keypoint pass ------------------------------
    def kp_chunk(c):
        pkc = io.tile([128, FKP], FP32, tag="pkc", name="pkc")
        gkc = io.tile([128, FKP], FP32, tag="gkc", name="gkc")
        nc.sync.dma_start(pkc[:], pk[c])
        nc.sync.dma_start(gkc[:], gk[c])
        cs, ce_ = c * SCH, (c + 1) * SCH
        # deinterleave (s k d) -> (d s k) into persistent bf16 (ACT copies)
        for src, dst in ((pkc, pball), (gkc, gball)):
            sv = src[:].rearrange("p (s k d) -> p d s k", s=SCH, k=SEC, d=3)
            for d in range(3):
                nc.scalar.activation(dst[:, d, cs:ce_], sv[:, d], AF.Copy)
        pbc = pball[:, :, cs:ce_]
        gbc = gball[:, :, cs:ce_]
        # keypoint smooth-l1
        dt = wk.tile([128, FKP], BF16, tag="dt", name="dt")
        nc.vector.tensor_tensor(dt[:], pbc, gbc, OP.subtract)
        ab = wk.tile([128, FKP], BF16, tag="ut", name="ab", bufs=1)
        nc.scalar.activation(ab[:], dt[:], AF.Abs)
        nc.scalar.activation(ab[:], ab[:], AF.Relu, bias=neg1[:])
        nc.scalar.activation(junk[:], dt[:], AF.Square,
                             accum_out=acc_a[:, CA_KPD2 + c: CA_KPD2 + c + 1])
        nc.scalar.activation(junk[:], ab[:], AF.Square,
                             accum_out=acc_a[:, CA_KPR2 + c: CA_KPR2 + c + 1])
        # raw H products (one row i at a time) + in-place tree + reduce
        prods = wk.tile([128, 3 * SCH * SEC], BF16, tag="prods", name="prods",
                        bufs=1)
        pv = prods[:].rearrange("p (n s k) -> p n s k", n=3, s=SCH)
        for i in range(3):
            for j in range(3):
                eng = nc.gpsimd if j == 1 else nc.vector
                eng.tensor_tensor(pv[:, j], gbc[:, i], pbc[:, j], OP.mult)
            nc.vector.tensor_tensor(pv[:, :, :, 0:10], pv[:, :, :, 0:10],
                                    pv[:, :, :, 10:20], OP.add)
            nc.vector.tensor_tensor(pv[:, :, :, 0:5], pv[:, :, :, 0:5],
                                    pv[:, :, :, 5:10], OP.add)
            nc.vector.tensor_reduce(hv[:, 3 * i:3 * i + 3, cs:ce_],
                                    pv[:, :, :, 0:5], axis=AX.X, op=OP.add)
        # per-(d,section) point sums, tree in prods scratch
        for srcv, base in ((pbc, 9), (gbc, 12)):
            stv = prods[:, :3 * SCH * 10].rearrange(
                "p (d s k) -> p d s k", d=3, s=SCH, k=10)
            nc.vector.tensor_tensor(stv, srcv[:, :, :, 0:10],
                                    srcv[:, :, :, 10:20], OP.add)
            nc.vector.tensor_tensor(stv[:, :, :, 0:5], stv[:, :, :, 0:5],
                                    stv[:, :, :, 5:10], OP.add)
            nc.vector.tensor_reduce(hv[:, base:base + 3, cs:ce_],
                                    stv[:, :, :, 0:5], axis=AX.X, op=OP.add)

    # ---------------- center loss + H correction + X0 ---------------------
    def center_corr_x0():
        spv = hv[:, 9:12]
        sgv = hv[:, 12:15]
        sps = pol.tile([128, 3 * SFD], FP32, tag="sps", name="sps")
        nc.vector.tensor_scalar(sps[:], spv, 1.0 / SEC, None, OP.mult)
        # center diff (sp - sg)/SEC and its smooth-l1 partials
        dc = pol.tile([128, 3 * SFD], FP32, tag="dc", name="dc")
        nc.vector.scalar_tensor_tensor(
            dc[:], sgv, -1.0 / SEC, sps[:].rearrange("p (d s) -> p d s", d=3),
            OP.mult, OP.add)
        cab = pol.tile([128, 3 * SFD], FP32, tag="cu", name="cab")
        nc.scalar.activation(cab[:], dc[:], AF.Abs)
        nc.scalar.activation(cab[:], cab[:], AF.Relu, bias=neg1[:])
        nc.scalar.activation(junk[:, :3 * SFD], dc[:], AF.Square,
                             accum_out=acc_a[:, CA_CTD2: CA_CTD2 + 1])
        nc.scalar.activation(junk[:, :3 * SFD], cab[:], AF.Square,
                             accum_out=acc_a[:, CA_CTR2: CA_CTR2 + 1])
        m9 = pol.tile([128, 9 * SFD], BF16, tag="A", name="m9")
        nc.vector.tensor_tensor(
            m9[:],
            sgv.unsqueeze(2).broadcast_to([128, 3, 3, SFD]),
            sps[:].rearrange("p (d s) -> p d s", d=3).unsqueeze(1)
                .broadcast_to([128, 3, 3, SFD]),
            OP.mult)
        nc.vector.tensor_tensor(X9, hv[:, 0:9], m9[:], OP.subtract)
        dup()

    def dup():
        nc.gpsimd.tensor_copy(xdv[:, 0:3, 3:5], xdv[:, 0:3, 0:2])
        nc.gpsimd.tensor_copy(xdv[:, 3:5, 0:5], xdv[:, 0:2, 0:5])

    # ---------------- polar: scaled-Newton iteration ----------------------
    def polar_iter(it):
        V1 = xdv[:, 1:4, 1:4]
        V2 = xdv[:, 2:5, 2:5]
        V3 = xdv[:, 1:4, 2:5]
        V4 = xdv[:, 2:5, 1:4]
        m1 = pol.tile([128, 9 * SFD], BF16, tag="A", name="m1")
        m2 = pol.tile([128, 9 * SFD], BF16, tag="B", name="m2")
        C9 = pol.tile([128, 9 * SFD], BF16, tag="C9", name="C9")
        nc.vector.tensor_tensor(m1[:], V1, V2, OP.mult)
        nc.vector.tensor_tensor(m2[:], V3, V4, OP.mult)
        nc.vector.tensor_tensor(C9[:], m1[:], m2[:], OP.subtract)
        # det = sum_j X[0,j] * C[0,j]
        dp = pol.tile([128, 3 * SFD], BF16, tag="dp", name="dp")
        nc.vector.tensor_tensor(dp[:], xdv[:, 0, 0:3], C9[:, 0:3 * SFD], OP.mult)
        det = pol.tile([128, SFD], FP32, tag="det", name="det")
        nc.vector.tensor_reduce(det[:], dp[:].rearrange("p (j s) -> p s j", j=3),
                                axis=AX.X, op=OP.add)
        # Frobenius norms
        sq = pol.tile([128, 9 * SFD], BF16, tag="A", name="sq")
        nX2 = pol.tile([128, SFD], FP32, tag="nX2", name="nX2")
        nc.vector.tensor_tensor(sq[:], X9, X9, OP.mult)
        nc.vector.tensor_reduce(nX2[:], sq[:].rearrange("p (n s) -> p s n", n=9),
                                axis=AX.X, op=OP.add)
        sq2 = pol.tile([128, 9 * SFD], BF16, tag="B", name="sq2")
        nC2 = pol.tile([128, SFD], FP32, tag="nC2", name="nC2")
        nc.gpsimd.tensor_tensor(sq2[:], C9[:], C9[:], OP.mult)
        nc.vector.tensor_reduce(nC2[:], sq2[:].rearrange("p (n s) -> p s n", n=9),
                                axis=AX.X, op=OP.add)
        # Scale-invariant zeta in ln/exp domain with a RELATIVE det floor
        # (|det| >= 1e-3 ||X||^3), so near-singular sections stay bounded:
        #   X' = A X + B C,   A = 0.5 exp(0.25(la - lb - 2 lcc)),
        #   |B| = 0.5 exp(0.25(lb - la - 2 lcc)),  sign(B) = sign(det)
        la = pol.tile([128, SFD], FP32, tag="la", name="la")
        lb = pol.tile([128, SFD], FP32, tag="lb", name="lb")
        da = pol.tile([128, SFD], FP32, tag="da", name="da")
        lc = pol.tile([128, SFD], FP32, tag="lc", name="lc")
        nc.scalar.activation(la[:], nC2[:], AF.Ln, bias=c_eps[:])
        nc.scalar.activation(lb[:], nX2[:], AF.Ln, bias=c_eps[:])
        nc.scalar.activation(da[:], det[:], AF.Abs)
        nc.scalar.activation(lc[:], da[:], AF.Ln, bias=c_eps[:])
        flo = pol.tile([128, SFD], FP32, tag="flo", name="flo")
        nc.vector.tensor_scalar(flo[:], lb[:], 1.5, LN_FLOOR, OP.mult, OP.add)
        lcc = pol.tile([128, SFD], FP32, tag="lcc", name="lcc")
        nc.vector.tensor_tensor(lcc[:], lc[:], flo[:], OP.max)
        zza = pol.tile([128, SFD], FP32, tag="zza", name="zza")
        nc.vector.scalar_tensor_tensor(zza[:], lb[:], -1.0, la[:],
                                       OP.mult, OP.add)       # la - lb
        zzb = pol.tile([128, SFD], FP32, tag="zzb", name="zzb")
        nc.vector.scalar_tensor_tensor(zzb[:], lcc[:], -2.0, zza[:],
                                       OP.mult, OP.add)       # la-lb-2lcc
        nc.vector.scalar_tensor_tensor(zza[:], lcc[:], -2.0, zza[:],
                                       OP.mult, OP.subtract)  # -2lcc-(la-lb)
        hz = pol.tile([128, SFD], BF16, tag="hz", name="hz")
        nc.scalar.activation(hz[:], zzb[:], AF.Exp, scale=0.25, bias=c_lnh[:])
        eb = pol.tile([128, SFD], FP32, tag="eb", name="eb")
        nc.scalar.activation(eb[:], zza[:], AF.Exp, scale=0.25, bias=c_lnh[:])
        sgn = pol.tile([128, SFD], FP32, tag="sgn", name="sgn")
        nc.scalar.activation(sgn[:], det[:], AF.Sign)
        wh = pol.tile([128, SFD], BF16, tag="wh", name="wh")
        nc.vector.tensor_tensor(wh[:], eb[:], sgn[:], OP.mult)
        # X' = (0.5 zeta) X + (0.5/(zeta det)) C
        u1 = pol.tile([128, 9 * SFD], BF16, tag="A", name="u1")
        u2 = pol.tile([128, 9 * SFD], BF16, tag="B", name="u2")
        nc.vector.tensor_tensor(
            u1[:], X9, hz[:].unsqueeze(1).broadcast_to([128, 9, SFD]), OP.mult)
        nc.vector.tensor_tensor(
            u2[:], C9[:], wh[:].unsqueeze(1).broadcast_to([128, 9, SFD]), OP.mult)
        nc.vector.tensor_tensor(X9, u1[:], u2[:], OP.add)
        dup()

    # ---------------- v_j = (sum_i sp_i R_ij - sg_j) / SEC ----------------
    def vcalc():
        # normalize R to ||R||_F = sqrt(3): a no-op for converged (orthogonal)
        # sections, and bounds any non-converged near-singular section.
        sqf = pol.tile([128, 9 * SFD], BF16, tag="A", name="sqf")
        nc.vector.tensor_tensor(sqf[:], X9, X9, OP.mult)
        nXf = pol.tile([128, SFD], FP32, tag="nX2", name="nXf")
        nc.vector.tensor_reduce(nXf[:], sqf[:].rearrange("p (n s) -> p s n", n=9),
                                axis=AX.X, op=OP.add)
        lnf = pol.tile([128, SFD], FP32, tag="lb", name="lnf")
        nc.scalar.activation(lnf[:], nXf[:], AF.Ln, bias=c_eps[:])
        rn = pol.tile([128, SFD], BF16, tag="hz", name="rn")
        nc.scalar.activation(rn[:], lnf[:], AF.Exp, scale=-0.5, bias=c_sq3[:])
        nc.vector.tensor_tensor(
            X9, X9, rn[:].unsqueeze(1).broadcast_to([128, 9, SFD]), OP.mult)

        spv = hv[:, 9:12]
        sgv = hv[:, 12:15]
        spsb = pol.tile([128, 3 * SFD], BF16, tag="spsb", name="spsb")
        nc.vector.tensor_scalar(spsb[:], spv, 1.0 / SEC, None, OP.mult)
        T9 = pol.tile([128, 9 * SFD], BF16, tag="A", name="T9")
        nc.vector.tensor_tensor(
            T9[:], X9,
            spsb[:].rearrange("p (i s) -> p i s", i=3).unsqueeze(2)
                .broadcast_to([128, 3, 3, SFD]),
            OP.mult)
        vs0 = pol.tile([128, 3 * SFD], FP32, tag="vs0", name="vs0")
        nc.vector.tensor_reduce(
            vs0[:], T9[:].rearrange("p (i j s) -> p j s i", i=3, j=3),
            axis=AX.X, op=OP.add)
        nc.vector.scalar_tensor_tensor(vvv, sgv, -1.0 / SEC, vs0[:].rearrange(
            "p (j s) -> p j s", j=3), OP.mult, OP.add)

    # ---------------- phase 3: rotation residual --------------------------
    def rot_chunk(c):
        cs, ce_ = c * SCH, (c + 1) * SCH
        pbc = pball[:, :, cs:ce_]
        gbc = gball[:, :, cs:ce_]
        rf = wk.tile([128, FKP], BF16, tag="dt", name="rf")
        rfv = rf[:].rearrange("p (j s k) -> p j s k", j=3, s=SCH)
        for j in range(3):
            rj = rfv[:, j]
            Rb = [X9[:, i, j, cs:ce_].unsqueeze(2).broadcast_to([128, SCH, SEC])
                  for i in range(3)]
            ma = wk.tile([128, SCH * SEC], BF16, tag="ma", name="ma")
            mb = wk.tile([128, SCH * SEC], BF16, tag="mb", name="mb")
            mav = ma[:].rearrange("p (s k) -> p s k", s=SCH)
            mbv = mb[:].rearrange("p (s k) -> p s k", s=SCH)
            nc.vector.tensor_tensor(rj, pbc[:, 0], Rb[0], OP.mult)
            nc.gpsimd.tensor_tensor(mav, pbc[:, 1], Rb[1], OP.mult)
            nc.vector.tensor_tensor(mbv, pbc[:, 2], Rb[2], OP.mult)
            nc.vector.tensor_tensor(rj, rj, mbv, OP.add)
            nc.vector.tensor_tensor(rj, rj, mav, OP.add)
            vjb = vvv[:, j, cs:ce_].unsqueeze(2).broadcast_to([128, SCH, SEC])
            nc.gpsimd.tensor_tensor(rj, rj, vjb, OP.subtract)
        nc.vector.tensor_tensor(rf[:], rf[:], gbc, OP.subtract)
        ab = wk.tile([128, FKP], BF16, tag="ut", name="ab", bufs=1)
        nc.scalar.activation(ab[:], rf[:], AF.Abs)
        nc.scalar.activation(ab[:], ab[:], AF.Relu, bias=neg1[:])
        nc.scalar.activation(junk[:], rf[:], AF.Square,
                             accum_out=acc_a[:, CA_ROTD2 + c: CA_ROTD2 + c + 1])
        nc.scalar.activation(junk[:], ab[:], AF.Square,
                             accum_out=acc_a[:, CA_ROTR2 + c: CA_ROTR2 + c + 1])

    # ---------------- emission schedule -----------------------------------
    ci = iter(range(NCH_CE))
    for c in range(NCH_KP):
        kp_chunk(c)
        ce_chunk(next(ci))
        ce_chunk(next(ci))
    center_corr_x0()
    for it in range(4):
        polar_iter(it)
        ce_chunk(next(ci))
        ce_chunk(next(ci))
    vcalc()
    for c in range(NCH_KP):
        rot_chunk(c)
        ce_chunk(next(ci))
    # lse: one Ln + accumulate over the whole core (in-place on se_all)
    nc.scalar.activation(se_all[:], se_all[:], AF.Ln,
                         accum_out=acc_a[:, CA_LSE: CA_LSE + 1])
    nc.sync.dma_start(out_v, acc_v[:])
    nc.sync.dma_start(out_a, acc_a[:])


_CACHE = {}


def _build():
    if "nc" in _CACHE:
        return _CACHE["nc"]
    nc = bacc.Bacc("TRN2", target_bir_lowering=False, debug=False,
                   enable_asserts=False, num_devices=N_CORES)
    aps = {
        "pk": nc.dram_tensor("pk", [NCH_KP, 128, FKP], FP32,
                             kind="ExternalInput").ap(),
        "gk": nc.dram_tensor("gk", [NCH_KP, 128, FKP], FP32,
                             kind="ExternalInput").ap(),
        "lg": nc.dram_tensor("lg", [NCH_CE, 128, FCE], FP32,
                             kind="ExternalInput").ap(),
        "out_v": nc.dram_tensor("out_v", [128, NACC_V], FP32,
                                kind="ExternalOutput").ap(),
        "out_a": nc.dram_tensor("out_a", [128, NACC_A], FP32,
                                kind="ExternalOutput").ap(),
    }
    with tile.TileContext(nc) as tc:
        with ExitStack() as ctx:
            _emit(ctx, tc, aps)
    nc.compile()
    _CACHE["nc"] = nc
    return nc


def _shard_inputs(pred_keypoints, gt_keypoints, pred_section_logits,
                  gt_section_label):
    # keypoints: sample = ((core*NCH_KP + chunk)*SPC + slot)*128 + p
    pk = np.ascontiguousarray(
        np.asarray(pred_keypoints, dtype=np.float32)
        .reshape(N_CORES, NCH_KP, SPC, 128, K * 3)
        .transpose(0, 1, 3, 2, 4)
        .reshape(N_CORES, NCH_KP, 128, FKP))
    gk = np.ascontiguousarray(
        np.asarray(gt_keypoints, dtype=np.float32)
        .reshape(N_CORES, NCH_KP, SPC, 128, K * 3)
        .transpose(0, 1, 3, 2, 4)
        .reshape(N_CORES, NCH_KP, 128, FKP))
    # logits: swap the label logit into slot 0 (lse is permutation-invariant)
    ls = np.asarray(pred_section_logits, dtype=np.float32).reshape(-1, NS).copy()
    lab = np.asarray(gt_section_label).reshape(-1).astype(np.int64)
    rows = np.arange(ls.shape[0])
    ly = ls[rows, lab].copy()
    ls[rows, lab] = ls[:, 0]
    ls[:, 0] = ly
    lgs = ls.reshape(N_CORES, NCH_CE, 128, FCE)
    return [
        {"pk": pk[i], "gk": gk[i], "lg": lgs[i]}
        for i in range(N_CORES)
    ]


def combine_accs(results):
    """results: list of (out_v [128,NACC_V], out_a [128,NACC_A]) per core."""
    tv = np.zeros(NACC_V, dtype=np.float64)
    ta = np.zeros(NACC_A, dtype=np.float64)
    for rv, ra in results:
        tv += rv.astype(np.float64).sum(axis=0)
        ta += ra.astype(np.float64).sum(axis=0)

    ce_sum = ta[CA_LSE] - tv[CV_LY:CV_LY + NCH_CE].sum()

    def sl1(d2, r2):
        return 0.5 * d2 - 0.5 * r2

    kp_sum = sl1(ta[CA_KPD2:CA_KPD2 + NCH_KP].sum(),
                 ta[CA_KPR2:CA_KPR2 + NCH_KP].sum())
    rot_sum = sl1(ta[CA_ROTD2:CA_ROTD2 + NCH_KP].sum(),
                  ta[CA_ROTR2:CA_ROTR2 + NCH_KP].sum())
    cent_sum = sl1(ta[CA_CTD2], ta[CA_CTR2])
    total = (1.0 * ce_sum / (B * K)
             + 4.0 * kp_sum / (B * K * 3)
             + 5.0 * rot_sum / (B * K * 3)
             + 6.0 * cent_sum / (B * S * 3))
    return np.float32(total)


def kernel(**inputs) -> np.ndarray:
    nc = _build()
    in_maps = _shard_inputs(**inputs)
    res = run_bass_kernel_spmd(nc, in_maps, list(range(N_CORES))).results
    return combine_accs([(res[i]["out_v"], res[i]["out_a"])
                         for i in range(N_CORES)])


# revision 28
# speedup vs baseline: 3.4582x; 1.1853x over previous
"""Trainium2 Bass kernel for nn_KPLoss_377957122199.

loss = 1*cross_entropy + 4*smoothL1(kp) + 5*smoothL1(Procrustes rot)
     + 6*smoothL1(section centers)

Data-parallel over 8 NeuronCores: batch 8192 -> 1024 per core; each core
emits per-partition partial sums, host combines.

Design notes (v2, instruction-count-optimized):
  * smooth_l1 sums via  sum f(d) = 0.5*sum d^2 - 0.5*sum u^2 + sum u - N/2,
    u = max(|d|,1):  ONE 4x-mode DVE tensor_scalar(abs_max)+accum and two
    ACT Square+accum per batch of elements.
  * keypoints deinterleaved (s k d)->(d s k) ONCE into persistent bf16
    SBUF tiles via 6 strided ACT copies per chunk; both the H-product pass
    and the rotation-residual pass read them (no second HBM load).
  * per-section 3x3 Gram matrices H via 9 bf16 2x DVE muls + an in-place
    pairwise add tree + one strided reduce per chunk.
  * polar decomposition R = polar(H) by Frobenius-scaled Newton iteration
    on a [128, 5,5,160] bf16 tile with duplicated rows/cols, so all 9
    cofactors come from THREE whole-matrix tensor_tensors on affine views.
    The zeta scaling runs in ln/exp domain so the only ACT functions the
    whole kernel needs (Copy/Square/Abs/Ln/Exp) live in ONE table set.
  * cross entropy: host swaps logits[t, y_t] <-> logits[t, 0] while
    sharding, so sum(l_y) is a strided-view reduce; lse = Ln(tree-summed
    exp) accumulated once over the whole core at the end.
  * CE chunks are emission-interleaved with the keypoint/polar phases so
    the logit DMA (the bulk of HBM traffic) prefetches throughout.
"""

import sys
for _p in ("/opt/trn_rl_repo", "/root/.axon_site/_ro/trn_rl_repo"):
    if _p not in sys.path:
        sys.path.insert(0, _p)

import math
from contextlib import ExitStack

import numpy as np

import concourse.bass as bass
import concourse.bacc as bacc
import concourse.mybir as mybir
import concourse.tile as tile
from concourse.bass_utils import run_bass_kernel_spmd

FP32 = mybir.dt.float32
BF16 = mybir.dt.bfloat16
AX = mybir.AxisListType
OP = mybir.AluOpType
AF = mybir.ActivationFunctionType

N_CORES = 8
B, K, NS, SEC = 8192, 400, 20, 20
S = K // SEC                      # 20 sections per sample
BC = B // N_CORES                 # 1024 samples per core
SPP = BC // 128                   # 8 samples per partition
SFD = SPP * S                     # 160 sections per partition
NCH_KP = 4                        # keypoint chunks
SPC = SPP // NCH_KP               # 2 samples per partition per chunk
FKP = SPC * K * 3                 # 2400 fp32 per partition per chunk
SCH = SPC * S                     # 40 sections per partition per chunk
NCH_CE = 20                       # cross-entropy chunks
TCE = BC * K // (NCH_CE * 128)    # 160 tokens per partition per chunk
FCE = TCE * NS                    # 3200 logits per partition per chunk
N_KP = BC * K * 3                 # smooth-l1 element counts (per core)
N_CENT = BC * S * 3

# acc column maps.  acc_v: DVE-written, acc_a: ACT-written (separate tiles
# so the two engines never touch the same SBUF region).
CV_LY = 0                         # NCH_CE cols
NACC_V = CV_LY + NCH_CE           # 20

NCH_ROT = 2                       # rotation-residual chunks (x3 j-planes)
CA_LSE = 0
CA_KPD2 = 1
CA_KPR2 = CA_KPD2 + NCH_KP
CA_ROTD2 = CA_KPR2 + NCH_KP
CA_ROTR2 = CA_ROTD2 + 3 * NCH_ROT
CA_CTD2 = CA_ROTR2 + 3 * NCH_ROT
CA_CTR2 = CA_CTD2 + 1
NACC_A = CA_CTR2 + 1              # 23

LN_HALF = float(math.log(0.5))
LN_FLOOR = float(math.log(1e-3))      # relative det floor: |det| >= 1e-3 ||X||^3
LN_SQRT3 = float(0.5 * math.log(3.0))
N_POLAR_ITERS = 3


def _emit(ctx: ExitStack, tc: "tile.TileContext", aps: dict):
    nc = tc.nc
    pk, gk, lg = aps["pk"], aps["gk"], aps["lg"]
    out_v, out_a = aps["out_v"], aps["out_a"]

    pers = ctx.enter_context(tc.tile_pool(name="pers", bufs=1))
    io = ctx.enter_context(tc.tile_pool(name="io", bufs=2))
    wk = ctx.enter_context(tc.tile_pool(name="wk", bufs=2))
    pol = ctx.enter_context(tc.tile_pool(name="pol", bufs=1))
    cep = ctx.enter_context(tc.tile_pool(name="ce", bufs=2))

    acc_v = pers.tile([128, NACC_V], FP32, tag="accv", name="accv")
    acc_a = pers.tile([128, NACC_A], FP32, tag="acca", name="acca")
    pb = pers.tile([128, 3 * SFD * SEC], BF16, tag="pb", name="pb")    # [p,d,s,k]
    gb = pers.tile([128, 3 * SFD * SEC], BF16, tag="gb", name="gb")
    hsp = pers.tile([128, 15 * SFD], FP32, tag="hsp", name="hsp")      # 0-8 H, 9-11 sp, 12-14 sg
    xd = pers.tile([128, 25 * SFD], BF16, tag="xd", name="xd")         # [p,r5,c5,s]
    vv = pers.tile([128, 3 * SFD], BF16, tag="vv", name="vv")          # [p,j,s]
    rb2 = pers.tile([128, 9 * SFD * 2], BF16, tag="rb2", name="rb2")   # [p,ij,s,2]
    vb2 = pers.tile([128, 3 * SFD * 2], BF16, tag="vb2", name="vb2")   # [p,j,s,2]
    se_all = pers.tile([128, NCH_CE * TCE], BF16, tag="se", name="se")

    pball = pb[:].rearrange("p (d s k) -> p d s k", d=3, s=SFD, k=SEC)
    gball = gb[:].rearrange("p (d s k) -> p d s k", d=3, s=SFD, k=SEC)
    hv = hsp[:].rearrange("p (n s) -> p n s", n=15)
    xdv = xd[:].rearrange("p (r c s) -> p r c s", r=5, c=5)
    X9 = xdv[:, 0:3, 0:3]                                              # [p,3,3,SFD]
    vvv = vv[:].rearrange("p (j s) -> p j s", j=3)
    rb2v = rb2[:].rearrange("p (n s j) -> p n s j", n=9, j=2)
    vb2v = vb2[:].rearrange("p (n s j) -> p n s j", n=3, j=2)

    c_eps = pers.tile([128, 1], FP32, tag="ceps", name="ceps")
    nc.gpsimd.memset(c_eps[:], 1e-12)
    c_lnh = pers.tile([128, 1], FP32, tag="clnh", name="clnh")
    nc.gpsimd.memset(c_lnh[:], LN_HALF)
    neg1 = pers.tile([128, 1], FP32, tag="neg1", name="neg1")
    nc.gpsimd.memset(neg1[:], -1.0)
    c_sq3 = pers.tile([128, 1], FP32, tag="csq3", name="csq3")
    nc.gpsimd.memset(c_sq3[:], LN_SQRT3)

    junk = pers.tile([128, FKP], BF16, tag="junk", name="junk")

    # ---------------- cross-entropy chunk (emission-interleaved) ----------
    def ce_chunk(ci):
        lgc = cep.tile([128, FCE], FP32, tag="lgc", name="lgc")
        nc.gpsimd.dma_start(lgc[:], lg[ci])
        # sum of slot-0 logits (host swapped the label logit into slot 0)
        lyv = lgc[:].rearrange("p (t c) -> p c t", c=NS)[:, 0]
        nc.vector.tensor_reduce(acc_v[:, CV_LY + ci: CV_LY + ci + 1], lyv,
                                axis=AX.X, op=OP.add)
        ex = cep.tile([128, FCE], BF16, tag="ex", name="ex", bufs=1)
        nc.scalar.activation(ex[:], lgc[:], AF.Exp)
        exv = ex[:].rearrange("p (t c) -> p t c", t=TCE)
        nc.vector.tensor_tensor(exv[:, :, 0:10], exv[:, :, 0:10],
                                exv[:, :, 10:20], OP.add)
        with nc.allow_low_precision(reason="bf16 per-token exp-sums; the "
                                    "final Ln accumulation is fp32"):
            nc.vector.tensor_reduce(se_all[:, ci * TCE:(ci + 1) * TCE],
                                    exv[:, :, 0:10], axis=AX.X, op=OP.add)

    # ---------------- phase 1: keypoint pass ------------------------------
    def kp_chunk(c):
        pkc = io.tile([128, FKP], FP32, tag="pkc", name="pkc")
        gkc = io.tile([128, FKP], FP32, tag="gkc", name="gkc")
        nc.sync.dma_start(pkc[:], pk[c])
        nc.sync.dma_start(gkc[:], gk[c])
        cs, ce_ = c * SCH, (c + 1) * SCH
        # deinterleave (s k d) -> (d s k) into persistent bf16 (ACT copies)
        for src, dst in ((pkc, pball), (gkc, gball)):
            sv = src[:].rearrange("p (s k d) -> p d s k", s=SCH, k=SEC, d=3)
            for d in range(3):
                nc.scalar.activation(dst[:, d, cs:ce_], sv[:, d], AF.Copy)
        pbc = pball[:, :, cs:ce_]
        gbc = gball[:, :, cs:ce_]
        # keypoint smooth-l1
        dt = wk.tile([128, FKP], BF16, tag="dt", name="dt")
        nc.vector.tensor_tensor(dt[:], pbc, gbc, OP.subtract)
        ab = wk.tile([128, FKP], BF16, tag="ut", name="ab", bufs=1)
        nc.scalar.activation(ab[:], dt[:], AF.Abs)
        nc.scalar.activation(ab[:], ab[:], AF.Relu, bias=neg1[:])
        nc.scalar.activation(junk[:], dt[:], AF.Square,
                             accum_out=acc_a[:, CA_KPD2 + c: CA_KPD2 + c + 1])
        nc.scalar.activation(junk[:], ab[:], AF.Square,
                             accum_out=acc_a[:, CA_KPR2 + c: CA_KPR2 + c + 1])
        # raw H products (one row i at a time) + in-place tree + reduce
        prods = wk.tile([128, 3 * SCH * SEC], BF16, tag="prods", name="prods",
                        bufs=1)
        pv = prods[:].rearrange("p (n s k) -> p n s k", n=3, s=SCH)
        for i in range(3):
            for j in range(3):
                nc.vector.tensor_tensor(pv[:, j], gbc[:, i], pbc[:, j], OP.mult)
            nc.vector.tensor_tensor(pv[:, :, :, 0:10], pv[:, :, :, 0:10],
                                    pv[:, :, :, 10:20], OP.add)
            nc.vector.tensor_tensor(pv[:, :, :, 0:5], pv[:, :, :, 0:5],
                                    pv[:, :, :, 5:10], OP.add)
            nc.vector.tensor_reduce(hv[:, 3 * i:3 * i + 3, cs:ce_],
                                    pv[:, :, :, 0:5], axis=AX.X, op=OP.add)
        # per-(d,section) point sums, tree in prods scratch
        for srcv, base in ((pbc, 9), (gbc, 12)):
            stv = prods[:, :3 * SCH * 10].rearrange(
                "p (d s k) -> p d s k", d=3, s=SCH, k=10)
            nc.vector.tensor_tensor(stv, srcv[:, :, :, 0:10],
                                    srcv[:, :, :, 10:20], OP.add)
            nc.vector.tensor_tensor(stv[:, :, :, 0:5], stv[:, :, :, 0:5],
                                    stv[:, :, :, 5:10], OP.add)
            nc.vector.tensor_reduce(hv[:, base:base + 3, cs:ce_],
                                    stv[:, :, :, 0:5], axis=AX.X, op=OP.add)

    # ---------------- center loss + H correction + X0 ---------------------
    def center_corr_x0():
        spv = hv[:, 9:12]
        sgv = hv[:, 12:15]
        sps = pol.tile([128, 3 * SFD], FP32, tag="sps", name="sps")
        nc.vector.tensor_scalar(sps[:], spv, 1.0 / SEC, None, OP.mult)
        # center diff (sp - sg)/SEC and its smooth-l1 partials
        dc = pol.tile([128, 3 * SFD], FP32, tag="dc", name="dc")
        nc.vector.scalar_tensor_tensor(
            dc[:], sgv, -1.0 / SEC, sps[:].rearrange("p (d s) -> p d s", d=3),
            OP.mult, OP.add)
        cab = pol.tile([128, 3 * SFD], FP32, tag="cu", name="cab")
        nc.scalar.activation(cab[:], dc[:], AF.Abs)
        nc.scalar.activation(cab[:], cab[:], AF.Relu, bias=neg1[:])
        nc.scalar.activation(junk[:, :3 * SFD], dc[:], AF.Square,
                             accum_out=acc_a[:, CA_CTD2: CA_CTD2 + 1])
        nc.scalar.activation(junk[:, :3 * SFD], cab[:], AF.Square,
                             accum_out=acc_a[:, CA_CTR2: CA_CTR2 + 1])
        m9 = pol.tile([128, 9 * SFD], BF16, tag="A", name="m9")
        nc.vector.tensor_tensor(
            m9[:],
            sgv.unsqueeze(2).broadcast_to([128, 3, 3, SFD]),
            sps[:].rearrange("p (d s) -> p d s", d=3).unsqueeze(1)
                .broadcast_to([128, 3, 3, SFD]),
            OP.mult)
        nc.vector.tensor_tensor(X9, hv[:, 0:9], m9[:], OP.subtract)
        dup()

    def dup():
        nc.vector.tensor_copy(xdv[:, 0:3, 3:5], xdv[:, 0:3, 0:2])
        nc.vector.tensor_copy(xdv[:, 3:5, 0:5], xdv[:, 0:2, 0:5])

    # ---------------- polar: scaled-Newton iteration ----------------------
    def polar_iter(it):
        V1 = xdv[:, 1:4, 1:4]
        V2 = xdv[:, 2:5, 2:5]
        V3 = xdv[:, 1:4, 2:5]
        V4 = xdv[:, 2:5, 1:4]
        m1 = pol.tile([128, 9 * SFD], BF16, tag="A", name="m1")
        m2 = pol.tile([128, 9 * SFD], BF16, tag="B", name="m2")
        C9 = pol.tile([128, 9 * SFD], BF16, tag="C9", name="C9")
        nc.vector.tensor_tensor(m1[:], V1, V2, OP.mult)
        nc.vector.tensor_tensor(m2[:], V3, V4, OP.mult)
        nc.vector.tensor_tensor(C9[:], m1[:], m2[:], OP.subtract)
        # det = sum_j X[0,j] * C[0,j]
        dp = pol.tile([128, 3 * SFD], BF16, tag="dp", name="dp")
        nc.vector.tensor_tensor(dp[:], xdv[:, 0, 0:3], C9[:, 0:3 * SFD], OP.mult)
        det = pol.tile([128, SFD], FP32, tag="det", name="det")
        nc.vector.tensor_reduce(det[:], dp[:].rearrange("p (j s) -> p s j", j=3),
                                axis=AX.X, op=OP.add)
        # Frobenius norms
        sq = pol.tile([128, 9 * SFD], BF16, tag="A", name="sq")
        nX2 = pol.tile([128, SFD], FP32, tag="nX2", name="nX2")
        nc.vector.tensor_tensor(sq[:], X9, X9, OP.mult)
        nc.vector.tensor_reduce(nX2[:], sq[:].rearrange("p (n s) -> p s n", n=9),
                                axis=AX.X, op=OP.add)
        sq2 = pol.tile([128, 9 * SFD], BF16, tag="B", name="sq2")
        nC2 = pol.tile([128, SFD], FP32, tag="nC2", name="nC2")
        nc.vector.tensor_tensor(sq2[:], C9[:], C9[:], OP.mult)
        nc.vector.tensor_reduce(nC2[:], sq2[:].rearrange("p (n s) -> p s n", n=9),
                                axis=AX.X, op=OP.add)
        # Scale-invariant zeta in ln/exp domain with a RELATIVE det floor
        # (|det| >= 1e-3 ||X||^3), so near-singular sections stay bounded:
        #   X' = A X + B C,   A = 0.5 exp(0.25(la - lb - 2 lcc)),
        #   |B| = 0.5 exp(0.25(lb - la - 2 lcc)),  sign(B) = sign(det)
        la = pol.tile([128, SFD], FP32, tag="la", name="la")
        lb = pol.tile([128, SFD], FP32, tag="lb", name="lb")
        da = pol.tile([128, SFD], FP32, tag="da", name="da")
        lc = pol.tile([128, SFD], FP32, tag="lc", name="lc")
        nc.scalar.activation(la[:], nC2[:], AF.Ln, bias=c_eps[:])
        nc.scalar.activation(lb[:], nX2[:], AF.Ln, bias=c_eps[:])
        nc.scalar.activation(da[:], det[:], AF.Abs)
        nc.scalar.activation(lc[:], da[:], AF.Ln, bias=c_eps[:])
        flo = pol.tile([128, SFD], FP32, tag="flo", name="flo")
        nc.vector.tensor_scalar(flo[:], lb[:], 1.5, LN_FLOOR, OP.mult, OP.add)
        lcc = pol.tile([128, SFD], FP32, tag="lcc", name="lcc")
        nc.vector.tensor_tensor(lcc[:], lc[:], flo[:], OP.max)
        zza = pol.tile([128, SFD], FP32, tag="zza", name="zza")
        nc.vector.scalar_tensor_tensor(zza[:], lb[:], -1.0, la[:],
                                       OP.mult, OP.add)       # la - lb
        zzb = pol.tile([128, SFD], FP32, tag="zzb", name="zzb")
        nc.vector.scalar_tensor_tensor(zzb[:], lcc[:], -2.0, zza[:],
                                       OP.mult, OP.add)       # la-lb-2lcc
        nc.vector.scalar_tensor_tensor(zza[:], lcc[:], -2.0, zza[:],
                                       OP.mult, OP.subtract)  # -2lcc-(la-lb)
        hz = pol.tile([128, SFD], BF16, tag="hz", name="hz")
        nc.scalar.activation(hz[:], zzb[:], AF.Exp, scale=0.25, bias=c_lnh[:])
        eb = pol.tile([128, SFD], FP32, tag="eb", name="eb")
        nc.scalar.activation(eb[:], zza[:], AF.Exp, scale=0.25, bias=c_lnh[:])
        sgn = pol.tile([128, SFD], FP32, tag="sgn", name="sgn")
        nc.scalar.activation(sgn[:], det[:], AF.Sign)
        wh = pol.tile([128, SFD], BF16, tag="wh", name="wh")
        nc.vector.tensor_tensor(wh[:], eb[:], sgn[:], OP.mult)
        # X' = (0.5 zeta) X + (0.5/(zeta det)) C
        u1 = pol.tile([128, 9 * SFD], BF16, tag="A", name="u1")
        u2 = pol.tile([128, 9 * SFD], BF16, tag="B", name="u2")
        nc.vector.tensor_tensor(
            u1[:], X9, hz[:].unsqueeze(1).broadcast_to([128, 9, SFD]), OP.mult)
        nc.vector.tensor_tensor(
            u2[:], C9[:], wh[:].unsqueeze(1).broadcast_to([128, 9, SFD]), OP.mult)
        nc.vector.tensor_tensor(X9, u1[:], u2[:], OP.add)
        dup()

    # ---------------- v_j = (sum_i sp_i R_ij - sg_j) / SEC ----------------
    def vcalc():
        # normalize R to ||R||_F = sqrt(3): a no-op for converged (orthogonal)
        # sections, and bounds any non-converged near-singular section.
        sqf = pol.tile([128, 9 * SFD], BF16, tag="A", name="sqf")
        nc.vector.tensor_tensor(sqf[:], X9, X9, OP.mult)
        nXf = pol.tile([128, SFD], FP32, tag="nX2", name="nXf")
        nc.vector.tensor_reduce(nXf[:], sqf[:].rearrange("p (n s) -> p s n", n=9),
                                axis=AX.X, op=OP.add)
        lnf = pol.tile([128, SFD], FP32, tag="lb", name="lnf")
        nc.scalar.activation(lnf[:], nXf[:], AF.Ln, bias=c_eps[:])
        rn = pol.tile([128, SFD], BF16, tag="hz", name="rn")
        nc.scalar.activation(rn[:], lnf[:], AF.Exp, scale=-0.5, bias=c_sq3[:])
        nc.vector.tensor_tensor(
            X9, X9, rn[:].unsqueeze(1).broadcast_to([128, 9, SFD]), OP.mult)

        spv = hv[:, 9:12]
        sgv = hv[:, 12:15]
        spsb = pol.tile([128, 3 * SFD], BF16, tag="spsb", name="spsb")
        nc.vector.tensor_scalar(spsb[:], spv, 1.0 / SEC, None, OP.mult)
        T9 = pol.tile([128, 9 * SFD], BF16, tag="A", name="T9")
        nc.vector.tensor_tensor(
            T9[:], X9,
            spsb[:].rearrange("p (i s) -> p i s", i=3).unsqueeze(2)
                .broadcast_to([128, 3, 3, SFD]),
            OP.mult)
        vs0 = pol.tile([128, 3 * SFD], FP32, tag="vs0", name="vs0")
        nc.vector.tensor_reduce(
            vs0[:], T9[:].rearrange("p (i j s) -> p j s i", i=3, j=3),
            axis=AX.X, op=OP.add)
        nc.vector.scalar_tensor_tensor(vvv, sgv, -1.0 / SEC, vs0[:].rearrange(
            "p (j s) -> p j s", j=3), OP.mult, OP.add)
        # pre-broadcast R and v to k=2 so the rotation-residual muls can use
        # interleaved [s, k10, k2] views with unit innermost stride (2x mode)
        for i in range(3):
            nc.scalar.activation(
                rb2v[:, 3 * i:3 * i + 3],
                xdv[:, i, 0:3].unsqueeze(3).broadcast_to([128, 3, SFD, 2]),
                AF.Copy)
        nc.scalar.activation(
            vb2v, vvv.unsqueeze(3).broadcast_to([128, 3, SFD, 2]), AF.Copy)

    # ---------------- phase 3: rotation residual --------------------------
    def rot_chunk(c):
        SC3 = SFD // NCH_ROT                        # 80 sections per chunk
        cs, ce_ = c * SC3, (c + 1) * SC3
        gbc = gball[:, :, cs:ce_]
        # [p, s, k] -> [p, s, k10, k2] interleaved views (k = 2*k10 + k2)
        def kv(t):
            return t.rearrange("p s (k j) -> p s k j", j=2)
        for j in range(3):
            rf = wk.tile([128, SC3 * SEC], BF16, tag="rf", name="rf", bufs=1)
            rfv = kv(rf[:].rearrange("p (s k) -> p s k", s=SC3))
            t3 = wk.tile([128, SC3 * SEC], BF16, tag="t3", name="t3", bufs=1)
            t3v = kv(t3[:].rearrange("p (s k) -> p s k", s=SC3))
            Rb = [rb2v[:, 3 * i + j, cs:ce_].unsqueeze(2)
                  .broadcast_to([128, SC3, SEC // 2, 2]) for i in range(3)]
            nc.vector.tensor_tensor(rfv, kv(pball[:, 0, cs:ce_]), Rb[0], OP.mult)
            nc.vector.tensor_tensor(t3v, kv(pball[:, 1, cs:ce_]), Rb[1], OP.mult)
            nc.vector.tensor_tensor(rf[:], rf[:], t3[:], OP.add)
            nc.vector.tensor_tensor(t3v, kv(pball[:, 2, cs:ce_]), Rb[2], OP.mult)
            nc.vector.tensor_tensor(rf[:], rf[:], t3[:], OP.add)
            vjb = vb2v[:, j, cs:ce_].unsqueeze(2).broadcast_to(
                [128, SC3, SEC // 2, 2])
            nc.vector.tensor_tensor(rfv, rfv, vjb, OP.subtract)
            nc.vector.tensor_tensor(rf[:], rf[:], gbc[:, j], OP.subtract)
            ab = wk.tile([128, SC3 * SEC], BF16, tag="ab3", name="ab3")
            nc.scalar.activation(ab[:], rf[:], AF.Abs)
            nc.scalar.activation(ab[:], ab[:], AF.Relu, bias=neg1[:])
            col = CA_ROTD2 + 3 * c + j
            nc.scalar.activation(junk[:, :SC3 * SEC], rf[:], AF.Square,
                                 accum_out=acc_a[:, col: col + 1])
            col = CA_ROTR2 + 3 * c + j
            nc.scalar.activation(junk[:, :SC3 * SEC], ab[:], AF.Square,
                                 accum_out=acc_a[:, col: col + 1])

    # ---------------- emission schedule -----------------------------------
    ci = iter(range(NCH_CE))
    for c in range(NCH_KP):
        kp_chunk(c)
        ce_chunk(next(ci))
        ce_chunk(next(ci))
    center_corr_x0()
    for it in range(N_POLAR_ITERS):
        polar_iter(it)
        ce_chunk(next(ci))
        ce_chunk(next(ci))
    vcalc()
    for c in range(NCH_ROT):
        rot_chunk(c)
        ce_chunk(next(ci))
        ce_chunk(next(ci))
        ce_chunk(next(ci))
    # lse: one Ln + accumulate over the whole core (in-place on se_all)
    nc.scalar.activation(se_all[:], se_all[:], AF.Ln,
                         accum_out=acc_a[:, CA_LSE: CA_LSE + 1])
    nc.sync.dma_start(out_v, acc_v[:])
    nc.sync.dma_start(out_a, acc_a[:])


_CACHE = {}


def _build():
    if "nc" in _CACHE:
        return _CACHE["nc"]
    nc = bacc.Bacc("TRN2", target_bir_lowering=False, debug=False,
                   enable_asserts=False, num_devices=N_CORES)
    aps = {
        "pk": nc.dram_tensor("pk", [NCH_KP, 128, FKP], FP32,
                             kind="ExternalInput").ap(),
        "gk": nc.dram_tensor("gk", [NCH_KP, 128, FKP], FP32,
                             kind="ExternalInput").ap(),
        "lg": nc.dram_tensor("lg", [NCH_CE, 128, FCE], FP32,
                             kind="ExternalInput").ap(),
        "out_v": nc.dram_tensor("out_v", [128, NACC_V], FP32,
                                kind="ExternalOutput").ap(),
        "out_a": nc.dram_tensor("out_a", [128, NACC_A], FP32,
                                kind="ExternalOutput").ap(),
    }
    with tile.TileContext(nc) as tc:
        with ExitStack() as ctx:
            _emit(ctx, tc, aps)
    nc.compile()
    _CACHE["nc"] = nc
    return nc


def _shard_inputs(pred_keypoints, gt_keypoints, pred_section_logits,
                  gt_section_label):
    # keypoints: sample = ((core*NCH_KP + chunk)*SPC + slot)*128 + p
    pk = np.ascontiguousarray(
        np.asarray(pred_keypoints, dtype=np.float32)
        .reshape(N_CORES, NCH_KP, SPC, 128, K * 3)
        .transpose(0, 1, 3, 2, 4)
        .reshape(N_CORES, NCH_KP, 128, FKP))
    gk = np.ascontiguousarray(
        np.asarray(gt_keypoints, dtype=np.float32)
        .reshape(N_CORES, NCH_KP, SPC, 128, K * 3)
        .transpose(0, 1, 3, 2, 4)
        .reshape(N_CORES, NCH_KP, 128, FKP))
    # logits: swap the label logit into slot 0 (lse is permutation-invariant)
    ls = np.asarray(pred_section_logits, dtype=np.float32).reshape(-1, NS).copy()
    lab = np.asarray(gt_section_label).reshape(-1).astype(np.int64)
    rows = np.arange(ls.shape[0])
    ly = ls[rows, lab].copy()
    ls[rows, lab] = ls[:, 0]
    ls[:, 0] = ly
    lgs = ls.reshape(N_CORES, NCH_CE, 128, FCE)
    return [
        {"pk": pk[i], "gk": gk[i], "lg": lgs[i]}
        for i in range(N_CORES)
    ]


def combine_accs(results):
    """results: list of (out_v [128,NACC_V], out_a [128,NACC_A]) per core."""
    tv = np.zeros(NACC_V, dtype=np.float64)
    ta = np.zeros(NACC_A, dtype=np.float64)
    for rv, ra in results:
        tv += rv.astype(np.float64).sum(axis=0)
        ta += ra.astype(np.float64).sum(axis=0)

    ce_sum = ta[CA_LSE] - tv[CV_LY:CV_LY + NCH_CE].sum()

    def sl1(d2, r2):
        return 0.5 * d2 - 0.5 * r2

    kp_sum = sl1(ta[CA_KPD2:CA_KPD2 + NCH_KP].sum(),
                 ta[CA_KPR2:CA_KPR2 + NCH_KP].sum())
    rot_sum = sl1(ta[CA_ROTD2:CA_ROTD2 + 3 * NCH_ROT].sum(),
                  ta[CA_ROTR2:CA_ROTR2 + 3 * NCH_ROT].sum())
    cent_sum = sl1(ta[CA_CTD2], ta[CA_CTR2])
    total = (1.0 * ce_sum / (B * K)
             + 4.0 * kp_sum / (B * K * 3)
             + 5.0 * rot_sum / (B * K * 3)
             + 6.0 * cent_sum / (B * S * 3))
    return np.float32(total)


def kernel(**inputs) -> np.ndarray:
    nc = _build()
    in_maps = _shard_inputs(**inputs)
    res = run_bass_kernel_spmd(nc, in_maps, list(range(N_CORES))).results
    return combine_accs([(res[i]["out_v"], res[i]["out_a"])
                         for i in range(N_CORES)])
